# revision 1
# baseline (speedup 1.0000x reference)
"""Trainium2 Bass kernel: transformer block (QKV proj + MHA + residual + LN +
MLP(relu) residual + LN) for B=2, S=4096, D=512, H=8.

Sharding: data-parallel over (batch, query-row-block) — 8 cores x 1024 query
rows. Each core recomputes K/V projections for its batch (4 cores share a
batch), attends over all 4096 keys, and runs the per-row tail. No cross-core
communication.

Layouts: feature-major ("T" = [d, rows]) so projection/attention matmuls chain
without re-transposing. Softmax sums come free from a ones-column appended to
each V tile. fp32r matmuls (full PE rate at moving-dim >= 256).
"""

import math

import numpy as np

import concourse.bass as bass
import concourse.mybir as mybir
import concourse.tile as tile
from concourse.masks import make_identity

# ---------------------------------------------------------------------------
# Workaround: this walrus build rejects >1 sync-wait on the TileContext exit
# drain (CoreV3 setupSyncWait "Too many sync wait commands"). Split the waits
# across single-wait NOPs.
_orig_drain_and_barrier = tile.TileContext._drain_and_barrier


def _split_drain_and_barrier(self, tick_clock, wait_clock):
    from concourse.tile import ScopedClock

    nc = self.nc
    drain_inst = nc.sync.drain()
    wait_clock.add_sem_waits(
        drain_inst.ins, ScopedClock({None: tick_clock.global_clock})
    )
    si = drain_inst.ins.sync_info
    waits = list(si.on_wait) if si and si.on_wait else []
    if len(waits) > 1:
        si.on_wait = waits[:1]
        for w in waits[1:]:
            nop = nc.sync.nop(nofuse=True, hint="drain_wait_split")
            nop.ins.sync_info = mybir.SyncInfo(on_wait=[w], on_update=[])
    nc.all_engine_barrier()
    assert self.sems is not None
    popped = nc._tile_sem_poison_stack.pop()
    assert popped is self._sem_poison
    nc.clear_and_free_semaphores(list(self.sems.allocated().values()))
    nc.all_engine_barrier()


tile.TileContext._drain_and_barrier = _split_drain_and_barrier


def _split_multi_waits(nc, limit=1):
    """walrus CoreV3 codegen caps sync-waits per instruction descriptor; hoist
    excess waits onto fresh NOPs inserted just before the instruction on the
    same engine queue. Matmul (S3_LW) and Drain (CTRL_NO) descriptors only
    fit 1 wait; other engine descriptors fit 2."""
    ctr = [0]

    def mknop(engine, wait):
        ctr[0] += 1
        nop = mybir.InstNoOp(name=f"WSPLIT-{ctr[0]}", ins=[], outs=[])
        nop.engine = engine
        nop.sync_info = mybir.SyncInfo(on_wait=[wait], on_update=[])
        return nop

    nsplit = 0
    for f in nc.m.functions:
        for bb in f.blocks:
            insts = bb.instructions
            i = 0
            while i < len(insts):
                ins = insts[i]
                si = getattr(ins, "sync_info", None)
                if si is not None and si.on_wait and len(si.on_wait) > limit:
                    waits = list(si.on_wait)
                    si.on_wait = waits[-limit:]
                    pre = [mknop(ins.engine, w) for w in waits[:-limit]]
                    for j, p in enumerate(pre):
                        insts.insert(i + j, p)
                        nc.register_instruction(p, overwrite=True)
                    i += len(pre)
                    nsplit += 1
                i += 1
    return nsplit, ctr[0]
# ---------------------------------------------------------------------------

B, S, D, H, DH = 2, 4096, 512, 8, 64
P = 128
NC = 8          # cores
RPC = 1024      # query rows per core
NKC = D // P    # 4 contraction chunks of 128
EPS = 1e-5
SCALE = 1.0 / math.sqrt(D)

F32 = mybir.dt.float32
F32R = mybir.dt.float32r
BF16 = mybir.dt.bfloat16
I32 = mybir.dt.int32
ALU = mybir.AluOpType
AF = mybir.ActivationFunctionType


def r(ap):
    """view an fp32 AP as float32r for full-rate PE matmuls"""
    return ap.bitcast(F32R)


def build_nc(reps=1):
    nc = bass.Bass()

    Qr = nc.dram_tensor("Qr", [RPC, D], F32, kind="ExternalInput")
    Kb = nc.dram_tensor("Kb", [S, D], F32, kind="ExternalInput")
    Wq = nc.dram_tensor("Wq", [D, D], F32R, kind="ExternalInput")
    Wk = nc.dram_tensor("Wk", [D, D], F32R, kind="ExternalInput")
    Wv = nc.dram_tensor("Wv", [D, D], F32R, kind="ExternalInput")
    Wo = nc.dram_tensor("Wo", [D, D], F32R, kind="ExternalInput")
    bq = nc.dram_tensor("bq", [D], F32, kind="ExternalInput")
    bk = nc.dram_tensor("bk", [D], F32, kind="ExternalInput")
    bv = nc.dram_tensor("bv", [D], F32, kind="ExternalInput")
    bo = nc.dram_tensor("bo", [D], F32, kind="ExternalInput")
    g0 = nc.dram_tensor("g0", [D], F32, kind="ExternalInput")
    b0 = nc.dram_tensor("b0", [D], F32, kind="ExternalInput")
    g1 = nc.dram_tensor("g1", [D], F32, kind="ExternalInput")
    b1 = nc.dram_tensor("b1", [D], F32, kind="ExternalInput")
    Oo = nc.dram_tensor("O", [RPC, D], F32, kind="ExternalOutput")

    def bcast_ap(dram_vec):
        # [cols] dram vector -> [P, cols] partition-broadcast AP
        a = dram_vec[:]
        return bass.AP(
            tensor=a.tensor,
            offset=a.offset,
            ap=[[0, P]] + list(a.ap),
        )

    def chunked_ap(dram_vec):
        # [D] dram vector -> [P, NKC]: partition = idx within 128-chunk
        return dram_vec.rearrange("(c p) -> p c", p=P)

    with tile.TileContext(nc) as tc:
        with (
            tc.tile_pool(name="consts", bufs=1) as consts,
            tc.tile_pool(name="kT_p", bufs=1) as kT_p,
            tc.tile_pool(name="vx_p", bufs=1) as vx_p,
            tc.tile_pool(name="qT_p", bufs=1) as qT_p,
            tc.tile_pool(name="w_p", bufs=3) as w_p,
            tc.tile_pool(name="ktc_p", bufs=2) as ktc_p,
            tc.tile_pool(name="kload_p", bufs=2) as kload_p,
            tc.tile_pool(name="at_p", bufs=4) as at_p,
            tc.tile_pool(name="ot_p", bufs=2) as ot_p,
            tc.tile_pool(name="opre_p", bufs=7) as opre_p,
            tc.tile_pool(name="tail_p", bufs=2) as tail_p,
            tc.tile_pool(name="stat_p", bufs=4) as stat_p,
            tc.tile_pool(name="ps_t", bufs=2, space="PSUM") as ps_t,
            tc.tile_pool(name="ps_big", bufs=3, space="PSUM") as ps_big,
        ):
          for _rep in range(reps):
            # ---- constants ----
            ident = consts.tile([P, P], F32)
            make_identity(nc, ident)
            eps_t = consts.tile([P, 1], F32)
            nc.vector.memset(eps_t, EPS)

            wq_t = w_p.tile([P, NKC, D], F32R, tag="w", name="wq_t")
            nc.sync.dma_start(wq_t, Wq.rearrange("(c p) n -> p c n", p=P))
            wk_t = w_p.tile([P, NKC, D], F32R, tag="w", name="wk_t")
            nc.sync.dma_start(wk_t, Wk.rearrange("(c p) n -> p c n", p=P))
            bias_sb = consts.tile([P, 3, NKC], F32)
            for wi, bvec in enumerate((bq, bk, bv)):
                nc.gpsimd.dma_start(bias_sb[:, wi], chunked_ap(bvec))
            bvb = consts.tile([P, H, DH], F32)
            bob = consts.tile([P, D], F32)
            g0b = consts.tile([P, D], F32)
            b0b = consts.tile([P, D], F32)
            g1b = consts.tile([P, D], F32)
            b1b = consts.tile([P, D], F32)
            nc.gpsimd.dma_start(bvb, bcast_ap(bv).rearrange("p (h d) -> p h d", h=H))
            for t, v in ((bob, bo), (g0b, g0), (b0b, b0), (g1b, g1), (b1b, b1)):
                nc.gpsimd.dma_start(t, bcast_ap(v))

            # ---- persistent activations ----
            kT = kT_p.tile([P, NKC, S], F32R)          # (K Wk + bk)^T
            v_ext = vx_p.tile([P, S // P, H, DH + 1], BF16)  # V rows + ones col
            qT = qT_p.tile([P, NKC, RPC], F32R)        # (Q Wq + bq)^T
            nc.vector.memset(v_ext[:, :, :, DH:DH + 1], 1.0)

            SC2 = 512
            RB = 512

            # ---- phase B: Q -> Q^T -> qT (+bq); rb0 half first ----
            def b_step(rc2):
                qtiles = []
                for half in range(SC2 // P):
                    qt = kload_p.tile([P, D], F32, tag="kload", name="qld")
                    nc.sync.dma_start(
                        qt, Qr[rc2 * SC2 + half * P: rc2 * SC2 + (half + 1) * P, :]
                    )
                    qtiles.append(qt)
                QTc = ktc_p.tile([P, NKC, SC2], F32R, tag="ktc", name="QTc")
                for half in range(SC2 // P):
                    for kc in range(NKC):
                        pt = ps_t.tile([P, P], F32, tag="pt", name="ptq")
                        nc.tensor.transpose(
                            pt, qtiles[half][:, kc * P:(kc + 1) * P], ident
                        )
                        nc.vector.tensor_copy(
                            QTc[:, kc, half * P:(half + 1) * P], pt
                        )
                for ci in range(NKC):
                    pp = ps_big.tile([P, SC2], F32, tag="big", name="ppq")
                    for kc in range(NKC):
                        nc.tensor.matmul(
                            pp,
                            lhsT=wq_t[:, kc, ci * P:(ci + 1) * P],
                            rhs=QTc[:, kc],
                            start=(kc == 0), stop=(kc == NKC - 1),
                        )
                    nc.vector.tensor_scalar_add(
                        qT[:, ci, rc2 * SC2:(rc2 + 1) * SC2], pp,
                        bias_sb[:, 0, ci:ci + 1],
                    )

            b_step(0)

            # ---- helpers ----
            def a_step(sc2):
                """project K rows [sc2*256, (sc2+1)*256) into kT and v_ext"""
                ktiles = []
                for half in range(SC2 // P):
                    kt = kload_p.tile([P, D], F32, tag="kload", name="kld")
                    nc.sync.dma_start(
                        kt, Kb[sc2 * SC2 + half * P: sc2 * SC2 + (half + 1) * P, :]
                    )
                    ktiles.append(kt)
                KTc = ktc_p.tile([P, NKC, SC2], F32R, tag="ktc", name="KTc")
                for half in range(SC2 // P):
                    for kc in range(NKC):
                        pt = ps_t.tile([P, P], F32, tag="pt", name="ptk")
                        nc.tensor.transpose(
                            pt, ktiles[half][:, kc * P:(kc + 1) * P], ident
                        )
                        nc.vector.tensor_copy(
                            KTc[:, kc, half * P:(half + 1) * P], pt
                        )
                for ci in range(NKC):
                    pp = ps_big.tile([P, SC2], F32, tag="big", name="ppk")
                    for kc in range(NKC):
                        nc.tensor.matmul(
                            pp,
                            lhsT=wk_t[:, kc, ci * P:(ci + 1) * P],
                            rhs=KTc[:, kc],
                            start=(kc == 0), stop=(kc == NKC - 1),
                        )
                    nc.vector.tensor_scalar_add(
                        kT[:, ci, sc2 * SC2:(sc2 + 1) * SC2], pp,
                        bias_sb[:, 1, ci:ci + 1],
                    )
                for half in range(SC2 // P):
                    pv = ps_big.tile([P, D], F32, tag="big", name="ppv")
                    for kc in range(NKC):
                        nc.tensor.matmul(
                            pv,
                            lhsT=KTc[:, kc, half * P:(half + 1) * P],
                            rhs=wv_t[:, kc],
                            start=(kc == 0), stop=(kc == NKC - 1),
                        )
                    sidx = sc2 * (SC2 // P) + half
                    nc.vector.scalar_tensor_tensor(
                        out=v_ext[:, sidx, :, 0:DH],
                        in0=pv.rearrange("p (h d) -> p h d", h=H),
                        scalar=1.0,
                        in1=bvb,
                        op0=ALU.mult, op1=ALU.add,
                    )

            def c_step(rb, pair, sc, obig):
                ps = ps_big.tile([P, 2, RB], F32, tag="big", name="psc")
                for i, h in enumerate((2 * pair, 2 * pair + 1)):
                    ci, po = h // 2, (h % 2) * DH
                    nc.tensor.matmul(
                        ps[:, i],
                        lhsT=kT[po:po + DH, ci, sc * P:(sc + 1) * P],
                        rhs=qT[po:po + DH, ci, rb * RB:(rb + 1) * RB],
                        start=True, stop=True,
                    )
                at = at_p.tile([P, 2, RB], BF16, tag="at", name="at")
                nc.scalar.activation(at, ps, AF.Exp, scale=SCALE)
                for i, h in enumerate((2 * pair, 2 * pair + 1)):
                    nc.tensor.matmul(
                        obig[0:DH + 1, i],
                        lhsT=v_ext[:, sc, h],
                        rhs=at[:, i],
                        start=(sc == 0), stop=(sc == S // P - 1),
                    )

            def c_post(rb, pair, obig, opre):
                for i, h in enumerate((2 * pair, 2 * pair + 1)):
                    ot = ot_p.tile([DH + 1, RB], F32, tag="ot", name="ot")
                    nc.vector.tensor_copy(ot, obig[0:DH + 1, i])
                    for rc in range(RB // P):
                        pt = ps_t.tile([P, P], F32, tag="pt", name="pto")
                        nc.tensor.transpose(
                            pt[:, 0:DH + 1], ot[:, rc * P:(rc + 1) * P],
                            ident[0:DH + 1, 0:DH + 1]
                        )
                        rec = stat_p.tile([P, 1], F32, tag="rec", name="rec")
                        nc.vector.reciprocal(rec, pt[:, DH:DH + 1])
                        nc.vector.tensor_scalar_mul(
                            opre[rc][:, h], pt[:, 0:DH], rec
                        )

            def layernorm(dst, src, gb, bb):
                st6 = stat_p.tile([P, 6], F32, tag="st6", name="st6")
                nc.vector.bn_stats(st6, src)
                mv = stat_p.tile([P, 2], F32, tag="mv", name="mv")
                nc.vector.bn_aggr(mv, st6)
                # rstd = rsqrt(var+eps) on DVE: quake seed + 2 Newton steps
                # (keeps ACT's table on Exp during the attention stream)
                srt = stat_p.tile([P, 1], F32, tag="srt", name="srt")
                nc.vector.tensor_scalar_add(srt, mv[:, 1:2], EPS)
                yv = stat_p.tile([P, 1], F32, tag="rstd", name="yv")
                yu = yv.bitcast(I32)
                nc.vector.tensor_scalar(yu, srt.bitcast(I32), 1, None,
                                        ALU.logical_shift_right)
                nc.vector.tensor_scalar(yu, yu, -1, None, ALU.bitwise_xor)
                nc.vector.tensor_scalar(yu, yu, 0x5F3759E0, None, ALU.add)
                rstd = yv
                for _it in range(2):
                    aa = stat_p.tile([P, 1], F32, tag="nsa", name="nsa")
                    nc.vector.tensor_mul(aa, rstd, rstd)
                    nc.vector.tensor_mul(aa, aa, srt)
                    nc.vector.tensor_scalar_add(aa, aa, -3.0)
                    nc.vector.scalar_tensor_tensor(
                        out=rstd, in0=rstd, scalar=-0.5, in1=aa,
                        op0=ALU.mult, op1=ALU.mult,
                    )
                xc = tail_p.tile([P, D], F32, tag="xc", name="xc")
                nc.vector.tensor_scalar_sub(xc, src, mv[:, 0:1])
                nc.vector.scalar_tensor_tensor(
                    out=dst, in0=xc, scalar=rstd, in1=gb,
                    op0=ALU.mult, op1=ALU.mult,
                )
                nc.vector.tensor_add(dst, dst, bb)

            def tail_rc(rb, rc, opre):
                gr = rb * (RB // P) + rc
                x = opre[rc].rearrange("p h d -> p (h d)")
                for kc in range(NKC):
                    pt = ps_t.tile([P, P], F32, tag="pt", name="ptr")
                    nc.tensor.transpose(
                        pt, qT.bitcast(F32)[:, kc, gr * P:(gr + 1) * P], ident
                    )
                    nc.vector.tensor_add(
                        x[:, kc * P:(kc + 1) * P],
                        x[:, kc * P:(kc + 1) * P], pt
                    )
                ln0 = tail_p.tile([P, D], F32, tag="ln0", name="ln0")
                layernorm(ln0, x, g0b, b0b)
                lnT = tail_p.tile([P, NKC, P], F32R, tag="lnT", name="lnT")
                for kc in range(NKC):
                    pt = ps_t.tile([P, P], F32, tag="pt", name="ptl")
                    nc.tensor.transpose(pt, ln0[:, kc * P:(kc + 1) * P], ident)
                    nc.vector.tensor_copy(lnT[:, kc], pt)
                pm = ps_big.tile([P, D], F32, tag="big", name="pmo")
                for kc in range(NKC):
                    nc.tensor.matmul(
                        pm, lhsT=lnT[:, kc], rhs=wo_t[:, kc],
                        start=(kc == 0), stop=(kc == NKC - 1),
                    )
                mlp = tail_p.tile([P, D], F32, tag="xc", name="mlp_t")
                nc.vector.scalar_tensor_tensor(
                    out=mlp, in0=pm, scalar=1.0, in1=bob,
                    op0=ALU.mult, op1=ALU.add,
                )
                nc.vector.tensor_scalar_max(mlp, mlp, 0.0)
                nc.vector.tensor_add(mlp, mlp, ln0)
                out_t = tail_p.tile([P, D], F32, tag="xc", name="out_t")
                layernorm(out_t, mlp, g1b, b1b)
                nc.sync.dma_start(Oo[gr * P:(gr + 1) * P, :], out_t)

            # ---- phase A interleaved with C(rb=0, pair=0) ----
            wv_t = w_p.tile([P, NKC, D], F32R, tag="w", name="wv_t")
            nc.sync.dma_start(wv_t, Wv.rearrange("(c p) n -> p c n", p=P))

            opre0 = [opre_p.tile([P, H, DH], F32, tag="opre", name=f"opre0_{i}")
                     for i in range(RB // P)]
            obig = ps_big.tile([P, 2, RB], F32, tag="big", name="oacc0")
            LAG = 1
            SPC = SC2 // P  # key-chunks produced per a_step
            for sc2 in range(S // SC2):
                a_step(sc2)
                if sc2 >= LAG:
                    for sc in range(SPC * (sc2 - LAG), SPC * (sc2 - LAG) + SPC):
                        c_step(0, 0, sc, obig)
            for sc in range(SPC * (S // SC2 - LAG), S // P):
                c_step(0, 0, sc, obig)
            c_post(0, 0, obig, opre0)

            # ---- remaining rb0 pairs (qT rb1-half emitted inside pair1) ----
            for pair in range(1, H // 2):
                obig = ps_big.tile([P, 2, RB], F32, tag="big", name=f"oacc_r0_p{pair}")
                for sc in range(S // P):
                    c_step(0, pair, sc, obig)
                    if pair == 1 and sc == 12:
                        b_step(1)
                c_post(0, pair, obig, opre0)

            wo_t = w_p.tile([P, NKC, D], F32R, tag="w", name="wo_t")
            nc.sync.dma_start(wo_t, Wo.rearrange("(c p) n -> p c n", p=P))

            # ---- rb1 pairs with rb0 tails spread into their PE slack ----
            opre1 = [opre_p.tile([P, H, DH], F32, tag="opre", name=f"opre1_{i}")
                     for i in range(RB // P)]
            for pair in range(H // 2):
                obig = ps_big.tile([P, 2, RB], F32, tag="big", name=f"oacc_r1_p{pair}")
                for sc in range(S // P):
                    c_step(1, pair, sc, obig)
                    if sc == 16:
                        tail_rc(0, pair, opre0)
                c_post(1, pair, obig, opre1)
            for rc in range(RB // P):
                tail_rc(1, rc, opre1)

    nsplit, nnops = _split_multi_waits(nc)
    print(f"wait-split: {nsplit} instructions, {nnops} nops inserted")
    return nc


_cached = {}


def _get_nc():
    if "nc" not in _cached:
        _cached["nc"] = build_nc()
    return _cached["nc"]


def kernel(Q, K, Wq, bq, Wk, bk, Wv, bv, Wo, bo, g0, b0, g1, b1):
    from concourse.bass_utils import run_bass_kernel_spmd

    nc = _get_nc()
    Q = np.ascontiguousarray(Q, dtype=np.float32)
    K = np.ascontiguousarray(K, dtype=np.float32)
    shared = {
        "Wq": np.ascontiguousarray(Wq, np.float32),
        "Wk": np.ascontiguousarray(Wk, np.float32),
        "Wv": np.ascontiguousarray(Wv, np.float32),
        "Wo": np.ascontiguousarray(Wo, np.float32),
        "bq": np.ascontiguousarray(bq, np.float32),
        "bk": np.ascontiguousarray(bk, np.float32),
        "bv": np.ascontiguousarray(bv, np.float32),
        "bo": np.ascontiguousarray(bo, np.float32),
        "g0": np.ascontiguousarray(g0, np.float32),
        "b0": np.ascontiguousarray(b0, np.float32),
        "g1": np.ascontiguousarray(g1, np.float32),
        "b1": np.ascontiguousarray(b1, np.float32),
    }
    in_maps = []
    for c in range(NC):
        b, roff = c // 4, (c % 4) * RPC
        in_maps.append(
            dict(shared,
                 Qr=np.ascontiguousarray(Q[b, roff:roff + RPC]),
                 Kb=np.ascontiguousarray(K[b]))
        )
    res = run_bass_kernel_spmd(nc, in_maps, core_ids=list(range(NC)))
    out = np.empty((B, S, D), np.float32)
    for c in range(NC):
        b, roff = c // 4, (c % 4) * RPC
        out[b, roff:roff + RPC] = res.results[c]["O"]
    return out



# revision 3
# speedup vs baseline: 1.1339x; 1.1339x over previous
"""Trainium2 Bass kernel v2: transformer block (QKV proj + MHA + residual + LN +
MLP(relu) residual + LN) for B=2, S=4096, D=512, H=8.

Sharding: data-parallel over (batch, query-row-block) - 8 cores x 1024 query
rows. Each core recomputes K/V projections for its batch, attends over all
4096 keys, runs the per-row tail. No cross-core communication.

v2 design (vs baseline):
- K/Q inputs cast fp32->bf16 via SWDGE DMA, transposed via DMA xbar (no PE
  transposes / DVE copies for input transposition).
- All projection matmuls in bf16 (full PE rate, FWL).
- Scores in fp8 (kT8/qT8), row-tiled 64x128 pairs (two heads concurrently).
- exp split between ACT (native Exp -> fp8) and DVE (Schraudolph bit-trick
  tensor_scalar -> uint8 == e4m3 bit pattern).
- exp'd scores staged in SBUF (at_sb, fp8) per (rb,pair) combo; AV matmuls in
  fp8 DoubleRow (256-key contraction pairs), ones-column gives softmax denom.
- Tail layernorms: DVE stats + batched Newton rsqrt, ACT normalize (fused
  per-partition scale/bias), GPSIMD applies per-feature gamma/beta, ACT relu.
"""

import math

import numpy as np

import concourse.bass as bass
import concourse.mybir as mybir
import concourse.tile as tile
from concourse.masks import make_identity
from concourse.tile_rust import add_dep_helper

# ---------------------------------------------------------------------------
# Workaround: this walrus build rejects >1 sync-wait on the TileContext exit
# drain. Split the waits across single-wait NOPs. (Same as baseline kernel.)
_orig_drain_and_barrier = tile.TileContext._drain_and_barrier


def _split_drain_and_barrier(self, tick_clock, wait_clock):
    from concourse.tile import ScopedClock

    nc = self.nc
    drain_inst = nc.sync.drain()
    wait_clock.add_sem_waits(
        drain_inst.ins, ScopedClock({None: tick_clock.global_clock})
    )
    si = drain_inst.ins.sync_info
    waits = list(si.on_wait) if si and si.on_wait else []
    if len(waits) > 1:
        si.on_wait = waits[:1]
        for w in waits[1:]:
            nop = nc.sync.nop(nofuse=True, hint="drain_wait_split")
            nop.ins.sync_info = mybir.SyncInfo(on_wait=[w], on_update=[])
    nc.all_engine_barrier()
    assert self.sems is not None
    popped = nc._tile_sem_poison_stack.pop()
    assert popped is self._sem_poison
    nc.clear_and_free_semaphores(list(self.sems.allocated().values()))
    nc.all_engine_barrier()


tile.TileContext._drain_and_barrier = _split_drain_and_barrier


def _split_multi_waits(nc, limit=1):
    """walrus CoreV3 codegen caps sync-waits per instruction descriptor; hoist
    excess waits onto fresh NOPs inserted just before the instruction on the
    same engine queue."""
    ctr = [0]

    def mknop(engine, wait):
        ctr[0] += 1
        nop = mybir.InstNoOp(name=f"WSPLIT-{ctr[0]}", ins=[], outs=[])
        nop.engine = engine
        nop.sync_info = mybir.SyncInfo(on_wait=[wait], on_update=[])
        return nop

    nsplit = 0
    for f in nc.m.functions:
        for bb in f.blocks:
            insts = bb.instructions
            i = 0
            while i < len(insts):
                ins = insts[i]
                si = getattr(ins, "sync_info", None)
                if si is not None and si.on_wait and len(si.on_wait) > limit:
                    waits = list(si.on_wait)
                    si.on_wait = waits[-limit:]
                    pre = [mknop(ins.engine, w) for w in waits[:-limit]]
                    for j, p in enumerate(pre):
                        insts.insert(i + j, p)
                        nc.register_instruction(p, overwrite=True)
                    i += len(pre)
                    nsplit += 1
                i += 1
    return nsplit, ctr[0]
# ---------------------------------------------------------------------------

B, S, D, H, DH = 2, 4096, 512, 8, 64
P = 128
NC = 8          # cores
RPC = 1024      # query rows per core
NKC = D // P    # 4 contraction chunks of 128
EPS = 1e-5
SCALE = 1.0 / math.sqrt(D)
LOG2E = 1.4426950408889634

F32 = mybir.dt.float32
BF16 = mybir.dt.bfloat16
FP8 = mybir.dt.float8e4
U8 = mybir.dt.uint8
I32 = mybir.dt.int32
ALU = mybir.AluOpType
AF = mybir.ActivationFunctionType
DR = mybir.MatmulPerfMode.DoubleRow

VW = DH + 2      # per-(sc,head) v2 row block: 64 v + 1 ones + 1 pad
assert (H * VW) % 16 == 0  # DoubleRow middle-dim step restriction

# exp engine split: fraction of score chunks on ACT, per phase (Bresenham)
import os as _os
ACT_FRAC_APHASE = float(_os.environ.get("K2_AF_A", "0.58"))
ACT_FRAC_CLOOP = float(_os.environ.get("K2_AF_C", "0.62"))
V2_ACT_FRAC = float(_os.environ.get("K2_V2A", "0.75"))
AT_BUFS = int(_os.environ.get("K2_ATB", "10"))
APH_COMBOS = int(_os.environ.get("K2_APC", "3"))
KIN8_MODE = _os.environ.get("K2_KIN8", "pool")

# Schraudolph constants for e4m3 bit pattern: u8 = round(x*A + B)
EXP_A = SCALE * LOG2E * 8.0
EXP_B = 7 * 8 - 0.58


def build_nc(reps=1):
    nc = bass.Bass()

    Qr = nc.dram_tensor("Qr", [RPC, D], F32, kind="ExternalInput")
    Kb = nc.dram_tensor("Kb", [S, D], F32, kind="ExternalInput")
    Wq = nc.dram_tensor("Wq", [D, D], F32, kind="ExternalInput")
    Wk = nc.dram_tensor("Wk", [D, D], F32, kind="ExternalInput")
    Wv = nc.dram_tensor("Wv", [D, D], F32, kind="ExternalInput")
    Wo = nc.dram_tensor("Wo", [D, D], F32, kind="ExternalInput")
    bq = nc.dram_tensor("bq", [D], F32, kind="ExternalInput")
    bk = nc.dram_tensor("bk", [D], F32, kind="ExternalInput")
    bv = nc.dram_tensor("bv", [D], F32, kind="ExternalInput")
    bo = nc.dram_tensor("bo", [D], F32, kind="ExternalInput")
    g0 = nc.dram_tensor("g0", [D], F32, kind="ExternalInput")
    b0 = nc.dram_tensor("b0", [D], F32, kind="ExternalInput")
    g1 = nc.dram_tensor("g1", [D], F32, kind="ExternalInput")
    b1 = nc.dram_tensor("b1", [D], F32, kind="ExternalInput")
    Oo = nc.dram_tensor("O", [RPC, D], F32, kind="ExternalOutput")

    Kb16 = nc.dram_tensor("Kb16", [S, D], BF16, kind="Internal")
    Qr16 = nc.dram_tensor("Qr16", [RPC, D], BF16, kind="Internal")

    def bcast_ap(dram_vec):
        a = dram_vec[:]
        return bass.AP(
            tensor=a.tensor,
            offset=a.offset,
            ap=[[0, P]] + list(a.ap),
        )

    def chunked_ap(dram_vec):
        # [D] dram vector -> [P, NKC]: partition = idx within 128-chunk
        return dram_vec.rearrange("(c p) -> p c", p=P)

    with tile.TileContext(nc) as tc:
        with (
            tc.tile_pool(name="consts", bufs=1) as consts,
            tc.tile_pool(name="persist", bufs=1) as persist,
            tc.tile_pool(name="kin_p", bufs=2) as kin_p,
            tc.tile_pool(name="at_p", bufs=AT_BUFS) as at_p,
            tc.tile_pool(name="opre_p", bufs=8) as opre_p,
            tc.tile_pool(name="ot_p", bufs=2) as ot_p,
            tc.tile_pool(name="tail_a", bufs=4) as tail_a,
            tc.tile_pool(name="tail_p", bufs=2) as tail_p,
            tc.tile_pool(name="stat_p", bufs=4) as stat_p,
            tc.tile_pool(name="ps_s", bufs=3, space="PSUM") as ps_s,
            tc.tile_pool(name="ps_o", bufs=1, space="PSUM") as ps_o,
        ):
          for _rep in range(reps):
            # ================= constants & weights =================
            # weights cast f32->bf16 during SWDGE DMA, d_in-major chunks
            wq16 = consts.tile([P, NKC, D], BF16, name="wq16")
            wk8 = consts.tile([P, NKC, D], FP8, name="wk8")
            wv8 = consts.tile([P, NKC, D], FP8, name="wv8")
            wo16 = consts.tile([P, NKC, D], BF16, name="wo16")
            # biases: per-partition chunked (for kT/qT evac), broadcast tiles
            bias_sb = consts.tile([P, 3, NKC], F32, name="bias_sb")
            for wi, bvec in enumerate((bq, bk, bv)):
                nc.gpsimd.dma_start(bias_sb[:, wi], chunked_ap(bvec))
            # softmax weights sum to 1, so A@(K Wv + bv) = A@(K Wv) + bv:
            # fold bv into the q-side bias applied in b_step.
            nc.vector.tensor_add(bias_sb[:, 0], bias_sb[:, 0], bias_sb[:, 2])
            bob = consts.tile([P, D], F32, name="bob")
            g0b = consts.tile([P, D], BF16, name="g0b")
            b0b = consts.tile([P, D], BF16, name="b0b")
            g1b = consts.tile([P, D], BF16, name="g1b")
            b1b = consts.tile([P, D], BF16, name="b1b")
            for t, v in ((bob, bo), (g0b, g0), (b0b, b0), (g1b, g1), (b1b, b1)):
                nc.gpsimd.dma_start(t, bcast_ap(v))

            identB = consts.tile([P, P], BF16, name="identB")
            make_identity(nc, identB)

            # input casts fp32 -> bf16 (DRAM->DRAM, SWDGE). DRAM tensors are
            # not dependency-tracked by the tile framework; record the cast
            # instructions and wire explicit deps to the transpose-loads.
            # Order: Q + Wq/Wk first (b_step(0) and the first a_step need
            # them), then K chunks, then remaining weights.
            CH = S // 8
            kcast = [None] * 8
            qcast = nc.gpsimd.dma_start(Qr16[:, :], Qr[:, :]).ins
            nc.gpsimd.dma_start(wq16, Wq.rearrange("(c p) n -> p c n", p=P))
            kcast[0] = nc.gpsimd.dma_start(Kb16[0:CH], Kb[0:CH]).ins
            nc.gpsimd.dma_start(wk8, Wk.rearrange("(c p) n -> p c n", p=P))
            kcast[1] = nc.gpsimd.dma_start(Kb16[CH:2 * CH], Kb[CH:2 * CH]).ins
            nc.gpsimd.dma_start(wv8, Wv.rearrange("(c p) n -> p c n", p=P))
            for j in range(2, 8):
                kcast[j] = nc.gpsimd.dma_start(
                    Kb16[j * CH:(j + 1) * CH], Kb[j * CH:(j + 1) * CH]).ins
            nc.gpsimd.dma_start(wo16, Wo.rearrange("(c p) n -> p c n", p=P))

            # ================= persistent activations =================
            kT8 = persist.tile([P, NKC, S], FP8, name="kT8")
            qT16 = persist.tile([P, NKC, RPC], BF16, name="qT16")
            qT8 = persist.tile([P, NKC, RPC], FP8, name="qT8")
            v2 = persist.tile([P, S // P, H, VW], FP8, name="v2")
            nc.vector.memset(v2[:, :, :, DH:DH + 1], 1.0)

            # transposed bf16 inputs (d_in-major)
            QT16 = persist.tile([P, NKC, RPC], BF16, name="QT16")
            for kc in range(NKC):
                ld = nc.sync.dma_start(
                    QT16[:, kc], Qr16[:, kc * P:(kc + 1) * P], transpose=True
                )
                add_dep_helper(ld.ins, qcast, True, "Qr16 cast before xpose load")

            # ================= phase helpers =================
            KBLK = 512           # keys per projection block
            NBLK = S // KBLK     # 8

            def kin_load(b):
                """DMA-transpose load of K block b -> [P, NKC, KBLK] bf16"""
                kin = kin_p.tile([P, NKC, KBLK], BF16, tag="kin", name=f"kin{b}")
                for kc in range(NKC):
                    ld = nc.sync.dma_start(
                        kin[:, kc],
                        Kb16[b * KBLK:(b + 1) * KBLK, kc * P:(kc + 1) * P],
                        transpose=True,
                    )
                    add_dep_helper(ld.ins, kcast[b * KBLK // CH], True,
                                   "Kb16 cast before xpose load")
                    if (b + 1) * KBLK > CH * (b * KBLK // CH + 1):
                        add_dep_helper(ld.ins, kcast[b * KBLK // CH + 1], True,
                                       "Kb16 cast before xpose load (tail)")
                kin8 = kin_p.tile([P, NKC, KBLK], FP8, tag="kin8", name=f"k8_{b}")
                if KIN8_MODE == "pool":
                    nc.gpsimd.tensor_copy(kin8, kin)
                else:
                    for kc in range(NKC):
                        if kc % 2 == 0:
                            nc.scalar.copy(kin8[:, kc], kin[:, kc])
                        else:
                            nc.vector.tensor_copy(kin8[:, kc], kin[:, kc])
                return kin8

            def a_step(b, kin):
                """project K block b into kT8 and v2 (fp8 DoubleRow)"""
                for ci in range(NKC):
                    pp = ps_s.tile([P, 2, KBLK], F32, tag="ps", name=f"ppk{b}{ci}")
                    for kj in range(NKC // 2):
                        nc.tensor.matmul(
                            pp[:, 0],
                            lhsT=wk8[:, 2 * kj:2 * kj + 2, ci * P:(ci + 1) * P],
                            rhs=kin[:, 2 * kj:2 * kj + 2],
                            start=(kj == 0), stop=(kj == NKC // 2 - 1),
                            perf_mode=DR,
                        )
                    if ci % 2 == 0:
                        nc.scalar.activation(
                            kT8[:, ci, b * KBLK:(b + 1) * KBLK], pp[:, 0],
                            AF.Identity, bias=bias_sb[:, 1, ci:ci + 1],
                            scale=1.0,
                        )
                    else:
                        nc.vector.tensor_scalar_add(
                            kT8[:, ci, b * KBLK:(b + 1) * KBLK], pp[:, 0],
                            bias_sb[:, 1, ci:ci + 1],
                        )
                # V-proj per 128-key chunk
                for half in range(KBLK // P):
                    pv = ps_s.tile([P, 2, KBLK], F32, tag="ps", name=f"ppv{b}{half}")
                    for kj in range(NKC // 2):
                        nc.tensor.matmul(
                            pv[:, 0, 0:D],
                            lhsT=kin[:, 2 * kj:2 * kj + 2, half * P:(half + 1) * P],
                            rhs=wv8[:, 2 * kj:2 * kj + 2],
                            start=(kj == 0), stop=(kj == NKC // 2 - 1),
                            perf_mode=DR,
                        )
                    sidx = b * (KBLK // P) + half
                    if (sidx % 4) < 4 * V2_ACT_FRAC:
                        nc.scalar.copy(
                            v2[:, sidx, :, 0:DH],
                            pv[:, 0, 0:D].rearrange("p (h d) -> p h d", h=H),
                        )
                    else:
                        nc.vector.tensor_copy(
                            v2[:, sidx, :, 0:DH],
                            pv[:, 0, 0:D].rearrange("p (h d) -> p h d", h=H),
                        )

            def b_step(rb):
                """project Q rows [rb*512, (rb+1)*512) -> qT16 + qT8"""
                for ci in range(NKC):
                    pp = ps_s.tile([P, 2, 512], F32, tag="ps", name=f"ppq{rb}{ci}")
                    for kc in range(NKC):
                        nc.tensor.matmul(
                            pp[:, 0],
                            lhsT=wq16[:, kc, ci * P:(ci + 1) * P],
                            rhs=QT16[:, kc, rb * 512:(rb + 1) * 512],
                            start=(kc == 0), stop=(kc == NKC - 1),
                        )
                    nc.vector.tensor_scalar_add(
                        qT16[:, ci, rb * 512:(rb + 1) * 512], pp[:, 0],
                        bias_sb[:, 0, ci:ci + 1],
                    )
                    nc.scalar.activation(
                        qT8[:, ci, rb * 512:(rb + 1) * 512], pp[:, 0],
                        AF.Identity, bias=bias_sb[:, 0, ci:ci + 1], scale=1.0,
                    )

            # ---- attention c-loop ----
            RB = 512
            SGRP = 8                     # score chunks per at-staging group
            NGRP = (S // P) // SGRP      # 4 groups per combo

            exp_acc = {"v": 0.0}

            def s_chunk(rb, pair, sc, at_sb, act_frac=ACT_FRAC_CLOOP):
                """scores for key chunk sc (both heads, row-tiled) + exp"""
                ps = ps_s.tile([P, 2, RB], F32, tag="ps", name=f"s{rb}{pair}{sc}")
                for i in range(2):
                    po = i * DH
                    nc.tensor.matmul(
                        ps[:, i],
                        lhsT=kT8[po:po + DH, pair, sc * P:(sc + 1) * P],
                        rhs=qT8[po:po + DH, pair, rb * RB:(rb + 1) * RB],
                        start=True, stop=True,
                    )
                exp_acc["v"] += act_frac
                if exp_acc["v"] >= 1.0:
                    exp_acc["v"] -= 1.0
                    nc.scalar.activation(at_sb[:, sc % SGRP], ps, AF.Exp,
                                         scale=SCALE)
                else:
                    nc.vector.tensor_scalar(
                        at_sb[:, sc % SGRP].bitcast(U8), ps, EXP_A, EXP_B,
                        ALU.mult, ALU.add,
                    )

            def v_group(rb, pair, g, at_sb, obig):
                """AV DoubleRow matmuls for group g (SGRP chunks)"""
                for c in range(SGRP // 2):
                    for i in range(2):
                        h = 2 * pair + i
                        nc.tensor.matmul(
                            obig[0:DH + 1, i],
                            lhsT=v2[:, g * SGRP + 2 * c:g * SGRP + 2 * c + 2,
                                    h, 0:DH + 1],
                            rhs=at_sb[:, 2 * c:2 * c + 2, i, :],
                            start=(g == 0 and c == 0),
                            stop=(g == NGRP - 1 and c == SGRP // 2 - 1),
                            perf_mode=DR,
                        )

            def c_post(rb, pair, obig, opre):
                """obig [65,2,RB] psum -> normalized attn out into opre"""
                ot = ot_p.tile([DH + 1, 2, RB], BF16, tag="ot", name="ot")
                if pair % 2 == 0:
                    nc.vector.tensor_copy(ot, obig[0:DH + 1])
                else:
                    nc.scalar.copy(ot, obig[0:DH + 1])
                for i in range(2):
                    h = 2 * pair + i
                    pt = ps_s.tile([P, 2, RB], F32, tag="ps", name=f"pt{pair}{i}")
                    ptb = pt.bitcast(BF16)  # transposes write bf16
                    ptv = ptb[:, 0, 0:(RB // P) * (DH + 2)].rearrange(
                        "p (rc w) -> p rc w", w=DH + 2
                    )
                    for rc in range(RB // P):
                        nc.tensor.transpose(
                            ptv[:, rc, 0:DH + 1],
                            ot[:, i, rc * P:(rc + 1) * P],
                            identB[0:DH + 1, 0:DH + 1],
                        )
                    rec = stat_p.tile([P, RB // P], F32, tag="rec", name="rec")
                    nc.vector.reciprocal(rec, ptv[:, :, DH])
                    for rc in range(RB // P):
                        nc.vector.tensor_scalar_mul(
                            opre[rc][:, h],
                            ptv[:, rc, 0:DH],
                            rec[:, rc:rc + 1],
                        )

            # ---- tail: residual + LN + MLP + LN over one rb (4 chunks) ----
            def tail_rb(rb, opre, part):
                """part 0: residual+stats+normalize ln0; runs per rc chunk"""
                rc = part
                gr = rb * (RB // P) + rc
                # residual: q rows (transpose qT16 back) + opre
                ptq = ps_s.tile([P, 2, RB], F32, tag="ps", name=f"ptq{rb}{rc}")
                ptqb = ptq.bitcast(BF16)
                for kc in range(NKC):
                    nc.tensor.transpose(
                        ptqb[:, 0, kc * P:(kc + 1) * P],
                        qT16[:, kc, gr * P:(gr + 1) * P],
                        identB,
                    )
                x = tail_a.tile([P, D], BF16, tag="x", name=f"x{rb}{rc}")
                nc.vector.tensor_add(
                    x, opre[rc].rearrange("p h d -> p (h d)"), ptqb[:, 0, 0:D]
                )
                return x

            def newton_rstd(var_eps, n):
                """rsqrt via int-seed + 2 Newton steps on DVE; [P, n] tiles"""
                yv = stat_p.tile([P, n], F32, tag="rstd", name="yv")
                yu = yv.bitcast(I32)
                nc.vector.tensor_scalar(yu, var_eps.bitcast(I32), 1, None,
                                        ALU.logical_shift_right)
                nc.vector.tensor_scalar(yu, yu, -1, None, ALU.bitwise_xor)
                nc.vector.tensor_scalar(yu, yu, 0x5F3759E0, None, ALU.add)
                for _it in range(2):
                    aa = stat_p.tile([P, n], F32, tag="nsa", name="nsa")
                    nc.vector.tensor_mul(aa, yv, yv)
                    nc.vector.tensor_mul(aa, aa, var_eps)
                    nc.vector.tensor_scalar_add(aa, aa, -3.0)
                    nc.vector.scalar_tensor_tensor(
                        out=yv, in0=yv, scalar=-0.5, in1=aa,
                        op0=ALU.mult, op1=ALU.mult,
                    )
                return yv

            def ln_stats1(x, tagn):
                """[P,D] tile -> (nmu [P,1], rstd [P,1])"""
                st6 = stat_p.tile([P, 6], F32, tag="st6", name=f"st{tagn}")
                nc.vector.bn_stats(st6, x)
                mv = stat_p.tile([P, 2], F32, tag="mv", name=f"mv{tagn}")
                nc.vector.bn_aggr(mv, st6)
                ve = stat_p.tile([P, 1], F32, tag="ve", name=f"ve{tagn}")
                nc.vector.tensor_scalar_add(ve, mv[:, 1:2], EPS)
                rstd = newton_rstd(ve, 1)
                nmu = stat_p.tile([P, 1], F32, tag="nmu", name=f"nm{tagn}")
                nc.vector.tensor_mul(nmu, mv[:, 0:1], rstd)
                nc.vector.tensor_scalar_mul(nmu, nmu, -1.0)
                return nmu, rstd

            def ln_apply1(dst, x, nmu, rstd, gb, bb):
                """dst = ((x - mu) * rstd) * g + b; ACT normalize + GPSIMD"""
                y = tail_p.tile([P, D], BF16, tag="y", name="y")
                nc.scalar.activation(
                    y, x, AF.Identity, bias=nmu[:, 0:1], scale=rstd[:, 0:1]
                )
                t = tail_p.tile([P, D], BF16, tag="t", name="t")
                nc.gpsimd.tensor_mul(t, y, gb)
                nc.gpsimd.tensor_add(dst, t, bb)

            def tail_batch(rb, opre, affine_dve=False):
                """stage-major tail over all 4 rc chunks; batched Newton"""
                def gmul(dst, y, gb):
                    if affine_dve:
                        nc.vector.scalar_tensor_tensor(
                            out=dst, in0=y, scalar=1.0, in1=gb,
                            op0=ALU.mult, op1=ALU.mult)
                    else:
                        nc.gpsimd.tensor_mul(dst, y, gb)

                def badd(dst, t, bb):
                    if affine_dve:
                        nc.vector.tensor_add(dst, t, bb)
                    else:
                        nc.gpsimd.tensor_add(dst, t, bb)

                n = RB // P
                xs = [tail_rb(rb, opre, rc) for rc in range(n)]
                mv = stat_p.tile([P, n, 2], F32, tag="mvB", name=f"mvB{rb}")
                for rc in range(n):
                    st6 = stat_p.tile([P, 6], F32, tag="st6", name=f"sB{rb}{rc}")
                    nc.vector.bn_stats(st6, xs[rc])
                    nc.vector.bn_aggr(mv[:, rc], st6)
                ve = stat_p.tile([P, n], F32, tag="veB", name=f"veB{rb}")
                nc.vector.tensor_scalar_add(ve, mv[:, :, 1], EPS)
                rstd = newton_rstd(ve, n)
                nmu = stat_p.tile([P, n], F32, tag="nmuB", name=f"nmB{rb}")
                nc.vector.tensor_mul(nmu, mv[:, :, 0], rstd)
                nc.vector.tensor_scalar_mul(nmu, nmu, -1.0)
                ln0s, x1s = [], []
                for rc in range(n):
                    ln0 = tail_a.tile([P, D], BF16, tag="ln0", name=f"lB{rb}{rc}")
                    y = tail_p.tile([P, D], BF16, tag="y", name="y")
                    nc.scalar.activation(y, xs[rc], AF.Identity,
                                         bias=nmu[:, rc:rc + 1],
                                         scale=rstd[:, rc:rc + 1])
                    t = tail_p.tile([P, D], BF16, tag="t", name="t")
                    gmul(t, y, g0b)
                    badd(ln0, t, b0b)
                    ln0s.append(ln0)
                for rc in range(n):
                    ptl = ps_s.tile([P, 2, RB], F32, tag="ps", name=f"pB{rb}{rc}")
                    ptlb = ptl.bitcast(BF16)
                    for kc in range(NKC):
                        nc.tensor.transpose(
                            ptlb[:, 0, kc * P:(kc + 1) * P],
                            ln0s[rc][:, kc * P:(kc + 1) * P],
                            identB,
                        )
                    lnT = tail_p.tile([P, NKC, P], BF16, tag="lnT", name="lnT")
                    nc.vector.tensor_copy(
                        lnT.rearrange("p c n -> p (c n)"), ptlb[:, 0, 0:D]
                    )
                    pm = ps_s.tile([P, 2, RB], F32, tag="ps", name=f"pmB{rb}{rc}")
                    for kc in range(NKC):
                        nc.tensor.matmul(
                            pm[:, 0], lhsT=lnT[:, kc], rhs=wo16[:, kc],
                            start=(kc == 0), stop=(kc == NKC - 1),
                        )
                    pmb = tail_p.tile([P, D], BF16, tag="pmb", name="pmb")
                    nc.vector.tensor_add(pmb, pm[:, 0], bob)
                    m16 = tail_p.tile([P, D], BF16, tag="m16", name="m16")
                    x1 = tail_a.tile([P, D], BF16, tag="x1", name=f"xB{rb}{rc}")
                    if affine_dve:
                        nc.vector.tensor_scalar_max(m16, pmb, 0.0)
                        nc.vector.tensor_add(x1, m16, ln0s[rc])
                    else:
                        nc.gpsimd.tensor_scalar_max(m16, pmb, 0.0)
                        nc.gpsimd.tensor_add(x1, m16, ln0s[rc])
                    x1s.append(x1)
                mv1 = stat_p.tile([P, n, 2], F32, tag="mvB", name=f"mvC{rb}")
                for rc in range(n):
                    st6 = stat_p.tile([P, 6], F32, tag="st6", name=f"sC{rb}{rc}")
                    nc.vector.bn_stats(st6, x1s[rc])
                    nc.vector.bn_aggr(mv1[:, rc], st6)
                ve1 = stat_p.tile([P, n], F32, tag="veB", name=f"veC{rb}")
                nc.vector.tensor_scalar_add(ve1, mv1[:, :, 1], EPS)
                rstd1 = newton_rstd(ve1, n)
                nmu1 = stat_p.tile([P, n], F32, tag="nmuB", name=f"nmC{rb}")
                nc.vector.tensor_mul(nmu1, mv1[:, :, 0], rstd1)
                nc.vector.tensor_scalar_mul(nmu1, nmu1, -1.0)
                for rc in range(n):
                    gr = rb * n + rc
                    y = tail_p.tile([P, D], BF16, tag="y", name="y")
                    nc.scalar.activation(y, x1s[rc], AF.Identity,
                                         bias=nmu1[:, rc:rc + 1],
                                         scale=rstd1[:, rc:rc + 1])
                    t = tail_p.tile([P, D], BF16, tag="t", name="t")
                    gmul(t, y, g1b)
                    out_t = tail_p.tile([P, D], F32, tag="outt", name="outt")
                    badd(out_t, t, b1b)
                    nc.sync.dma_start(Oo[gr * P:(gr + 1) * P, :], out_t)

            def tail_chunk(rb, rc, opre):
                x = tail_rb(rb, opre, rc)
                nmu0, rstd0 = ln_stats1(x, f"a{rb}{rc}")
                ln0 = tail_a.tile([P, D], BF16, tag="ln0", name=f"l0{rb}{rc}")
                ln_apply1(ln0, x, nmu0, rstd0, g0b, b0b)
                # Wo matmul on ln0^T
                ptl = ps_s.tile([P, 2, RB], F32, tag="ps", name=f"ptl{rb}{rc}")
                ptlb = ptl.bitcast(BF16)
                for kc in range(NKC):
                    nc.tensor.transpose(
                        ptlb[:, 0, kc * P:(kc + 1) * P],
                        ln0[:, kc * P:(kc + 1) * P],
                        identB,
                    )
                lnT = tail_p.tile([P, NKC, P], BF16, tag="lnT", name="lnT")
                nc.vector.tensor_copy(
                    lnT.rearrange("p c n -> p (c n)"), ptlb[:, 0, 0:D]
                )
                pm = ps_s.tile([P, 2, RB], F32, tag="ps", name=f"pm{rb}{rc}")
                for kc in range(NKC):
                    nc.tensor.matmul(
                        pm[:, 0], lhsT=lnT[:, kc], rhs=wo16[:, kc],
                        start=(kc == 0), stop=(kc == NKC - 1),
                    )
                pmb = tail_p.tile([P, D], BF16, tag="pmb", name="pmb")
                nc.vector.tensor_add(pmb, pm[:, 0], bob)
                m16 = tail_p.tile([P, D], BF16, tag="m16", name="m16")
                nc.gpsimd.tensor_scalar_max(m16, pmb, 0.0)
                x1 = tail_a.tile([P, D], BF16, tag="x1", name=f"x1{rb}{rc}")
                nc.gpsimd.tensor_add(x1, m16, ln0)
                nmu1, rstd1 = ln_stats1(x1, f"b{rb}{rc}")
                gr = rb * (RB // P) + rc
                out_t = tail_p.tile([P, D], F32, tag="outt", name="outt")
                ln_apply1(out_t, x1, nmu1, rstd1, g1b, b1b)
                nc.sync.dma_start(Oo[gr * P:(gr + 1) * P, :], out_t)

            # ================= schedule =================
            # Combo order interleaves both pair-0 combos (rb0 and rb1) with
            # the projection phase so exp density doubles there; rb1 finishes
            # at combo 6 (tail chunks spread into combo 7), rb0 tail batched
            # at the end.
            combos = [(0, 0), (1, 0), (1, 1), (0, 1),
                      (0, 2), (1, 2), (1, 3), (0, 3)]
            opre = {
                0: [opre_p.tile([P, H, DH], BF16, tag="opre", name=f"o0_{i}")
                    for i in range(RB // P)],
                1: [opre_p.tile([P, H, DH], BF16, tag="opre", name=f"o1_{i}")
                    for i in range(RB // P)],
            }
            at_tiles = {}
            obig_cur = [None]
            completed = []
            pos = {}
            vstate = {"next": 0}
            vlist = [(c, g) for c in range(len(combos)) for g in range(NGRP)]

            def emit_V(c, g):
                rb, pair = combos[c]
                if g == 0:
                    obig_cur[0] = ps_o.tile([P, 2, RB], F32, tag="obig",
                                            name=f"ob{c}")
                v_group(rb, pair, g, at_tiles[(c, g)], obig_cur[0])
                del at_tiles[(c, g)]
                if g == NGRP - 1:
                    c_post(rb, pair, obig_cur[0], opre[rb])

            def drain_v():
                while vstate["next"] < len(vlist):
                    c, g = vlist[vstate["next"]]
                    if (c, g) not in pos:
                        break
                    margin = 2 if len(completed) < 16 else 1
                    if pos[(c, g)] + margin > len(completed):
                        break
                    emit_V(c, g)
                    vstate["next"] += 1

            def s_group_completed(c, g):
                pos[(c, g)] = len(completed)
                completed.append((c, g))
                drain_v()

            b_step(0)
            b_step(1)

            # a-phase: K/V projection blocks interleaved with S-chunks of
            # combos 0-3 (pairs 0 and 1 of both rb) - the exp stream
            # saturates ACT/DVE from the first block onward
            kin = kin_load(0)
            for b in range(NBLK):
                kin_next = kin_load(b + 1) if b + 1 < NBLK else None
                a_step(b, kin)
                g = b // 2
                for c in range(APH_COMBOS):
                    if b % 2 == 0:
                        if (c, g) not in at_tiles:
                            at_tiles[(c, g)] = at_p.tile(
                                [P, SGRP, 2, RB], FP8, tag="at", name=f"at{c}_{g}")
                    rb, pair = combos[c]
                    for sc in range(b * (KBLK // P), (b + 1) * (KBLK // P)):
                        s_chunk(rb, pair, sc, at_tiles[(c, g)],
                                act_frac=ACT_FRAC_APHASE)
                    if b % 2 == 1:
                        s_group_completed(c, g)
                kin = kin_next

            for c in range(APH_COMBOS, len(combos)):
                rb, pair = combos[c]
                for g in range(NGRP):
                    at_tiles[(c, g)] = at_p.tile(
                        [P, SGRP, 2, RB], FP8, tag="at", name=f"at{c}_{g}")
                    for sc in range(g * SGRP, (g + 1) * SGRP):
                        s_chunk(rb, pair, sc, at_tiles[(c, g)])
                    s_group_completed(c, g)
                    if c == len(combos) - 1 and g >= 1:
                        tail_chunk(1, g - 1, opre[1])
            drain_v()
            tail_chunk(1, 3, opre[1])
            tail_batch(0, opre[0], affine_dve=True)

    nsplit, nnops = _split_multi_waits(nc)
    print(f"wait-split: {nsplit} instructions, {nnops} nops inserted")
    return nc


_cached = {}


def _get_nc():
    if "nc" not in _cached:
        _cached["nc"] = build_nc()
    return _cached["nc"]


def kernel(Q, K, Wq, bq, Wk, bk, Wv, bv, Wo, bo, g0, b0, g1, b1):
    from concourse.bass_utils import run_bass_kernel_spmd

    nc = _get_nc()
    Q = np.ascontiguousarray(Q, dtype=np.float32)
    K = np.ascontiguousarray(K, dtype=np.float32)
    shared = {
        "Wq": np.ascontiguousarray(Wq, np.float32),
        "Wk": np.ascontiguousarray(Wk, np.float32),
        "Wv": np.ascontiguousarray(Wv, np.float32),
        "Wo": np.ascontiguousarray(Wo, np.float32),
        "bq": np.ascontiguousarray(bq, np.float32),
        "bk": np.ascontiguousarray(bk, np.float32),
        "bv": np.ascontiguousarray(bv, np.float32),
        "bo": np.ascontiguousarray(bo, np.float32),
        "g0": np.ascontiguousarray(g0, np.float32),
        "b0": np.ascontiguousarray(b0, np.float32),
        "g1": np.ascontiguousarray(g1, np.float32),
        "b1": np.ascontiguousarray(b1, np.float32),
    }
    in_maps = []
    for c in range(NC):
        b, roff = c // 4, (c % 4) * RPC
        in_maps.append(
            dict(shared,
                 Qr=np.ascontiguousarray(Q[b, roff:roff + RPC]),
                 Kb=np.ascontiguousarray(K[b]))
        )
    res = run_bass_kernel_spmd(nc, in_maps, core_ids=list(range(NC)))
    out = np.empty((B, S, D), np.float32)
    for c in range(NC):
        b, roff = c // 4, (c % 4) * RPC
        out[b, roff:roff + RPC] = res.results[c]["O"]
    return out
